# revision 1
# baseline (speedup 1.0000x reference)
"""ClassicalSelfAttention TRN2 kernel — 8-core SPMD, sequence-parallel.

out = softmax((X Wq)(X Wk)^T / sqrt(d)) @ X,  X:[4096,1024] f32, W:[1024,1024].

Strategy (per core, rows sharded 8x512):
  A   = Wq @ Wk^T                (replicated, fp16x2 split matmuls)
  B^T = A^T-contraction form:  B^T[e,m] = sum_d A[d,e] Xl^T[d,m]   (fp16x2)
  S   = B @ X^T  via lhsT=B^T tiles, rhs=X^T chunks                (fp16x2)
  P   = softmax(S/32) row-wise (2-pass, ACT exp with per-partition bias)
  out = (P @ X) * (1/rowsum)   (fp32r matmuls, PSUM fp32 accumulate)

All transposes on PE (fp32, via identity), hi/lo fp16 split happens on the
PSUM->SBUF copy-out (DVE). Logit precision ~ fp32-grade (bf16 single-pass
flips argmax rows here; see softmax sharpness: top-2 logit gaps down to 3e-3).
P^T is spilled to DRAM and streamed back during PV to keep SBUF under budget.
"""
import numpy as np
import concourse.bass as bass
import concourse.bacc as bacc
import concourse.mybir as mybir
import concourse.tile as tile
from concourse import masks
from concourse.bass_utils import run_bass_kernel_spmd

F32 = mybir.dt.float32
F32R = mybir.dt.float32r
F16 = mybir.dt.float16

D = 1024          # embed dim
NT = 4096         # tokens
NC = 8            # cores
NL = NT // NC     # 512 local rows
DT = D // 128     # 8 d-tiles
JC = NT // 512    # 8 j-chunks
MT = NL // 128    # 4 m-tiles
SCALE = float(1.0 / np.sqrt(np.float32(D)))

EXP = mybir.ActivationFunctionType.Exp
COPY = mybir.ActivationFunctionType.Copy


def _split_copy(nc, psrc, hdst, ldst):
    """psum f32 -> hdst f16 (round) and ldst f16 (residual), both on DVE."""
    nc.vector.tensor_copy(hdst, psrc)
    nc.vector.tensor_sub(ldst, psrc, hdst)


def build_nc():
    nc = bacc.Bacc("TRN2", target_bir_lowering=False, debug=False)

    x_full = nc.declare_dram_parameter("x_full", [NT, D], F32, isOutput=False)
    x_local = nc.declare_dram_parameter("x_local", [NL, D], F32, isOutput=False)
    wq = nc.declare_dram_parameter("wq", [D, D], F32, isOutput=False)
    wk = nc.declare_dram_parameter("wk", [D, D], F32, isOutput=False)
    out_l = nc.declare_dram_parameter("out_local", [NL, D], F32, isOutput=True)
    pt_dram = nc.dram_tensor("pt_scratch", [NT, NL], F32R)

    with tile.TileContext(nc) as tc:
        with (
            tc.tile_pool(name="persist", bufs=1) as persist,
            tc.tile_pool(name="stream", bufs=6) as stream,
            tc.tile_pool(name="stats", bufs=1) as stats,
        ):
            ident = persist.tile([128, 128], F32, tag="ident", name="ident")
            masks.make_identity(nc, ident[:])

            # ---- stats tiles ----
            pmax = [stats.tile([128, JC], F32, tag=f"pmax{m}", name=f"pmax{m}") for m in range(MT)]
            esum = [stats.tile([128, JC], F32, tag=f"esum{m}", name=f"esum{m}") for m in range(MT)]
            recip = stats.tile([128, MT], F32, tag="recip", name="recip")

            with (
                tc.tile_pool(name="psum1", bufs=4, space=bass.MemorySpace.PSUM) as ptp,
                tc.tile_pool(name="psum2", bufs=4, space=bass.MemorySpace.PSUM) as pacc,
            ):
                # ---------- persistent mid-life tensors ----------
                with tc.tile_pool(name="abuf", bufs=1) as abuf:
                    A_h = [abuf.tile([128, D], F16, tag=f"Ah{r}", name=f"Ah{r}") for r in range(DT)]
                    A_l = [abuf.tile([128, D], F16, tag=f"Al{r}", name=f"Al{r}") for r in range(DT)]

                    # ---------- P1: W^T hi/lo ----------
                    with tc.tile_pool(name="wt", bufs=1) as wtp:
                        wT = {}
                        for wname in ("q", "k"):
                            for h in ("h", "l"):
                                wT[wname + h] = [
                                    wtp.tile([128, DT, 128], F16, tag=f"w{wname}{h}{c}", name=f"w{wname}{h}{c}")
                                    for c in range(DT)
                                ]
                        for wname, wdram in (("q", wq), ("k", wk)):
                            for r in range(DT):
                                wrow = stream.tile([128, D], F32, tag="row", name="wrow")
                                nc.sync.dma_start(wrow[:], wdram[r * 128:(r + 1) * 128, :])
                                for c in range(DT):
                                    pt = ptp.tile([128, 128], F32, tag="tp", name="tp")
                                    nc.tensor.transpose(pt[:], wrow[:, c * 128:(c + 1) * 128], ident[:])
                                    _split_copy(nc, pt[:], wT[wname + "h"][c][:, r, :], wT[wname + "l"][c][:, r, :])

                        # ---------- P2: A = Wq @ Wk^T  (fp16x2) ----------
                        for r in range(DT):
                            for ec in range(2):
                                pa = pacc.tile([128, 512], F32, tag="acc", name="acc")
                                n_mm = 0
                                for c in range(DT):
                                    for lh, rh in (("h", "h"), ("h", "l"), ("l", "h")):
                                        nc.tensor.matmul(
                                            pa[:],
                                            wT["q" + lh][c][:, r, :],
                                            wT["k" + rh][c][:, ec * 4:(ec + 1) * 4, :],
                                            start=(n_mm == 0), stop=(n_mm == 23),
                                        )
                                        n_mm += 1
                                _split_copy(nc, pa[:], A_h[r][:, ec * 512:(ec + 1) * 512],
                                            A_l[r][:, ec * 512:(ec + 1) * 512])

                    # ---------- P3: x_local^T hi/lo ----------
                    with tc.tile_pool(name="btbuf", bufs=1) as btbuf:
                        xlT_h = [btbuf.tile([128, MT, 128], F16, tag=f"xlTh{d}", name=f"xlTh{d}") for d in range(DT)]
                        xlT_l = [btbuf.tile([128, MT, 128], F16, tag=f"xlTl{d}", name=f"xlTl{d}") for d in range(DT)]
                        BT_h = [btbuf.tile([128, MT, 128], F16, tag=f"BTh{e}", name=f"BTh{e}") for e in range(DT)]
                        BT_l = [btbuf.tile([128, MT, 128], F16, tag=f"BTl{e}", name=f"BTl{e}") for e in range(DT)]

                        for js in range(MT):
                            xr = stream.tile([128, D], F32, tag="row", name="xrow")
                            nc.sync.dma_start(xr[:], x_local[js * 128:(js + 1) * 128, :])
                            for d in range(DT):
                                pt = ptp.tile([128, 128], F32, tag="tp", name="tp")
                                nc.tensor.transpose(pt[:], xr[:, d * 128:(d + 1) * 128], ident[:])
                                _split_copy(nc, pt[:], xlT_h[d][:, js, :], xlT_l[d][:, js, :])

                        # ---------- P4: B^T[e,m] = sum_d A[d,e] xlT[d,m] ----------
                        for e in range(DT):
                            pb = pacc.tile([128, 512], F32, tag="acc", name="acc")
                            n_mm = 0
                            for d in range(DT):
                                for lh, rh in (("h", "h"), ("h", "l"), ("l", "h")):
                                    lhsT = (A_h if lh == "h" else A_l)[d][:, e * 128:(e + 1) * 128]
                                    rhs = (xlT_h if rh == "h" else xlT_l)[d][:]
                                    nc.tensor.matmul(pb[:], lhsT, rhs,
                                                     start=(n_mm == 0), stop=(n_mm == 23))
                                    n_mm += 1
                            _split_copy(nc, pb[:], BT_h[e][:], BT_l[e][:])

                        # ---------- P5: S chunks + running max ----------
                        with (
                            tc.tile_pool(name="xt", bufs=2) as xtp,
                            tc.tile_pool(name="sbig", bufs=1) as sbig,
                            tc.tile_pool(name="pst", bufs=2) as pstp,
                        ):
                            S = [sbig.tile([128, JC, 512], F32, tag=f"S{m}", name=f"S{m}") for m in range(MT)]
                            for jc in range(JC):
                                xT_h = xtp.tile([128, DT, 512], F16, tag="xTh", name="xTh")
                                xT_l = xtp.tile([128, DT, 512], F16, tag="xTl", name="xTl")
                                for js in range(4):
                                    xr = stream.tile([128, D], F32, tag="row", name="xrow")
                                    nc.sync.dma_start(xr[:], x_full[jc * 512 + js * 128:jc * 512 + (js + 1) * 128, :])
                                    for d in range(DT):
                                        pt = ptp.tile([128, 128], F32, tag="tp", name="tp")
                                        nc.tensor.transpose(pt[:], xr[:, d * 128:(d + 1) * 128], ident[:])
                                        _split_copy(nc, pt[:], xT_h[:, d, js * 128:(js + 1) * 128],
                                                    xT_l[:, d, js * 128:(js + 1) * 128])
                                for m in range(MT):
                                    ps = pacc.tile([128, 512], F32, tag="acc", name="acc")
                                    n_mm = 0
                                    for e in range(DT):
                                        for lh, rh in (("h", "h"), ("h", "l"), ("l", "h")):
                                            lhsT = (BT_h if lh == "h" else BT_l)[e][:, m, :]
                                            rhs = (xT_h if rh == "h" else xT_l)[:, e, :]
                                            nc.tensor.matmul(ps[:], lhsT, rhs,
                                                             start=(n_mm == 0), stop=(n_mm == 23))
                                            n_mm += 1
                                    nc.scalar.activation(S[m][:, jc, :], ps[:], COPY)
                                    nc.vector.reduce_max(pmax[m][:, jc:jc + 1], ps[:],
                                                         axis=mybir.AxisListType.X)

                            # ---------- P6: softmax + P^T (spill to DRAM) ----------
                            for m in range(MT):
                                rowmax = stats.tile([128, 1], F32, tag=f"rmax{m}", name=f"rmax{m}")
                                nc.vector.reduce_max(rowmax[:], pmax[m][:],
                                                     axis=mybir.AxisListType.X)
                                negb = stats.tile([128, 1], F32, tag=f"negb{m}", name=f"negb{m}")
                                nc.vector.tensor_scalar_mul(negb[:], rowmax[:], -SCALE)
                                for jc in range(JC):
                                    pchunk = pstp.tile([128, 512], F32, tag="pchunk", name="pchunk")
                                    nc.scalar.activation(pchunk[:], S[m][:, jc, :], EXP,
                                                         bias=negb[:], scale=SCALE,
                                                         accum_out=esum[m][:, jc:jc + 1])
                                    ptst = pstp.tile([128, 4, 128], F32R, tag="ptst", name="ptst")
                                    for js in range(4):
                                        pt = ptp.tile([128, 128], F32, tag="tp", name="tp")
                                        nc.tensor.transpose(pt[:], pchunk[:, js * 128:(js + 1) * 128], ident[:])
                                        nc.vector.tensor_copy(ptst[:, js, :], pt[:])
                                    nc.sync.dma_start(
                                        pt_dram[jc * 512:(jc + 1) * 512, m * 128:(m + 1) * 128]
                                        .rearrange("(js p) m -> p js m", p=128),
                                        ptst[:],
                                    )
                                rs = stats.tile([128, 1], F32, tag=f"rs{m}", name=f"rs{m}")
                                nc.vector.reduce_sum(rs[:], esum[m][:], axis=mybir.AxisListType.X)
                                nc.vector.reciprocal(recip[:, m:m + 1], rs[:])

            # ---------- P7: out = (P @ V) * recip   (fp32r) ----------
            with tc.tile_pool(name="pv", bufs=1, space=bass.MemorySpace.PSUM) as pvp:
                with tc.tile_pool(name="ptin", bufs=6) as ptin, tc.tile_pool(name="p7s", bufs=6) as p7s:
                    ppv = [[pvp.tile([128, 512], F32, tag=f"pv{m}_{n}", name=f"pv{m}_{n}") for n in range(2)]
                           for m in range(MT)]
                    for jt in range(NT // 128):
                        vt = p7s.tile([128, D], F32R, tag="vt", name="vt")
                        nc.sync.dma_start(vt[:], x_full[jt * 128:(jt + 1) * 128, :].bitcast(F32R))
                        ptt = ptin.tile([128, NL], F32R, tag="ptt", name="ptt")
                        nc.sync.dma_start(ptt[:], pt_dram[jt * 128:(jt + 1) * 128, :])
                        for m in range(MT):
                            for n in range(2):
                                nc.tensor.matmul(
                                    ppv[m][n][:],
                                    ptt[:, m * 128:(m + 1) * 128],
                                    vt[:, n * 512:(n + 1) * 512],
                                    start=(jt == 0), stop=(jt == NT // 128 - 1),
                                )
                    for m in range(MT):
                        for n in range(2):
                            osb = p7s.tile([128, 512], F32, tag="osb", name="osb")
                            nc.vector.tensor_scalar_mul(osb[:], ppv[m][n][:], recip[:, m:m + 1])
                            nc.sync.dma_start(
                                out_l[m * 128:(m + 1) * 128, n * 512:(n + 1) * 512], osb[:])

    nc.compile()
    return nc


_NC_CACHE = None


def kernel(inputs, rotation_params, entangle_params):
    global _NC_CACHE
    if _NC_CACHE is None:
        _NC_CACHE = build_nc()
    nc = _NC_CACHE
    x = np.ascontiguousarray(np.asarray(inputs, np.float32))
    wq = np.ascontiguousarray(np.asarray(rotation_params, np.float32))
    wk = np.ascontiguousarray(np.asarray(entangle_params, np.float32))
    in_maps = [
        {"x_full": x, "x_local": x[c * NL:(c + 1) * NL], "wq": wq, "wk": wk}
        for c in range(NC)
    ]
    r = run_bass_kernel_spmd(nc, in_maps, list(range(NC)))
    return np.concatenate([r.results[c]["out_local"] for c in range(NC)], axis=0)



# revision 7
# speedup vs baseline: 1.8054x; 1.8054x over previous
"""ClassicalSelfAttention TRN2 kernel — 8-core SPMD, sequence-parallel.

out = softmax((X Wq)(X Wk)^T / sqrt(d)) @ X,  X:[4096,1024] f32, W:[1024,1024].

Per core (rows sharded 8x512), using S_l = ((Xl Wq) Wk^T) X^T so no
replicated projection work:
  Q^T = Wq^T Xl^T        fp16x2 (hh+hl+lh), lhsT = Wq natural layout
  C^T = Wk   Q^T         fp16x2, lhsT = Wk^T (host-transposed)
  S   = C    X^T         fp16x2, rhs = X^T (host-transposed + host-split)
  P   = softmax(S/32)    2-pass; S stored fp16 as (S - chunkmax)*scale
  out = (P @ X) * 1/rowsum   single-pass fp16 (P^T via PE transpose)

Host prep inside kernel(): transpose + fp16 hi/lo split of X / X^T / Wq /
Wk^T (layout-only work; all FLOPs of the computation run on device).
Logits need ~fp32 precision (top-2 gaps down to 3e-3 post-scale; bf16 or
f32r single-pass flips argmax rows), hence fp16x2 for the whole S chain.
P/V tolerate fp16 single-pass. P^T is consumed tile-by-tile straight out
of PSUM copies, so nothing spills to DRAM.
"""
import os
import numpy as np
import concourse.bass as bass
import concourse.bacc as bacc
import concourse.mybir as mybir
import concourse.tile as tile
from concourse import masks
from concourse.bass_utils import run_bass_kernel_spmd

F32 = mybir.dt.float32
F16 = mybir.dt.float16

D = 1024          # embed dim
NT = 4096         # tokens
NC = 8            # cores
NL = NT // NC     # 512 local rows
ET = D // 128     # 8 embed tiles
JC = NT // 512    # 8 j-chunks
MT = NL // 128    # 4 m-tiles
SCALE = float(1.0 / np.sqrt(np.float32(D)))

EXP = mybir.ActivationFunctionType.Exp
IDENT = mybir.ActivationFunctionType.Identity
AX = mybir.AxisListType.X
SUB = mybir.AluOpType.subtract
MUL = mybir.AluOpType.mult


def _split(nc, psrc, hdst, ldst):
    """psum f32 -> hdst f16 (round) and ldst f16 (residual), both on DVE."""
    nc.vector.tensor_copy(hdst, psrc)
    nc.vector.tensor_sub(ldst, psrc, hdst)


_PHASES = int(os.environ.get("KPHASES", "99"))


def build_nc():
    nc = bacc.Bacc("TRN2", target_bir_lowering=False, debug=False)

    xh_d = nc.declare_dram_parameter("xh", [NT, D], F16, isOutput=False)
    xth_d = nc.declare_dram_parameter("xth", [D, NT], F16, isOutput=False)
    xtl_d = nc.declare_dram_parameter("xtl", [D, NT], F16, isOutput=False)
    xlth_d = nc.declare_dram_parameter("xlth", [D, NL], F16, isOutput=False)
    xltl_d = nc.declare_dram_parameter("xltl", [D, NL], F16, isOutput=False)
    wqh_d = nc.declare_dram_parameter("wqh", [D, D], F16, isOutput=False)
    wql_d = nc.declare_dram_parameter("wql", [D, D], F16, isOutput=False)
    wkth_d = nc.declare_dram_parameter("wkth", [D, D], F16, isOutput=False)
    wktl_d = nc.declare_dram_parameter("wktl", [D, D], F16, isOutput=False)
    out_d = nc.declare_dram_parameter("out_local", [NL, D], F32, isOutput=True)

    with tile.TileContext(nc) as tc:
        with (
            tc.tile_pool(name="persist", bufs=1) as persist,
            tc.tile_pool(name="stats", bufs=1) as stats,
        ):
            ident16 = persist.tile([128, 128], F16, tag="id16", name="id16")
            masks.make_identity(nc, ident16[:])

            # chunk max / exp-pass bias / exp sums, one [128, JC] per m-tile
            pmax = [stats.tile([128, JC], F32, tag=f"pmax{m}", name=f"pmax{m}") for m in range(MT)]
            bsub = [stats.tile([128, JC], F32, tag=f"bsub{m}", name=f"bsub{m}") for m in range(MT)]
            bexp = [stats.tile([128, JC], F32, tag=f"bexp{m}", name=f"bexp{m}") for m in range(MT)]
            esum = [stats.tile([128, JC], F32, tag=f"esum{m}", name=f"esum{m}") for m in range(MT)]
            recip = stats.tile([128, MT], F32, tag="recip", name="recip")

            cth = persist.tile([128, ET, NL], F16, tag="cth", name="cth")
            ctl = persist.tile([128, ET, NL], F16, tag="ctl", name="ctl")
            s16 = [persist.tile([128, JC, 512], F16, tag=f"s16_{m}", name=f"s16_{m}")
                   for m in range(MT)]
            xh_sb = persist.tile([128, NT // 128, D], F16, tag="xhsb", name="xhsb")

            # ---------------- P1 + P2: Q^T then C^T ----------------
            with tc.tile_pool(name="psA", bufs=4, space=bass.MemorySpace.PSUM) as psA:
                with tc.tile_pool(name="qt", bufs=1) as qtp:
                    qth = qtp.tile([128, ET, NL], F16, tag="qth", name="qth")
                    qtl = qtp.tile([128, ET, NL], F16, tag="qtl", name="qtl")

                    with tc.tile_pool(name="w1", bufs=1) as w1:
                        wqh_sb = w1.tile([128, ET, D], F16, tag="wqh", name="wqh")
                        wql_sb = w1.tile([128, ET, D], F16, tag="wql", name="wql")
                        xlth_sb = w1.tile([128, ET, NL], F16, tag="xlth", name="xlth")
                        xltl_sb = w1.tile([128, ET, NL], F16, tag="xltl", name="xltl")
                        # per-es pieces so the first matmuls can start early
                        for es in range(ET):
                            r = slice(es * 128, (es + 1) * 128)
                            nc.sync.dma_start(xlth_sb[:, es, :], xlth_d[r, :])
                            nc.sync.dma_start(xltl_sb[:, es, :], xltl_d[r, :])
                            nc.sync.dma_start(wqh_sb[:, es, :], wqh_d[r, :])
                            nc.sync.dma_start(wql_sb[:, es, :], wql_d[r, :])

                        for dp in range(ET if _PHASES >= 1 else 0):
                            pq = psA.tile([128, NL], F32, tag="acc", name="acc")
                            n_mm = 0
                            for es in range(ET):
                                dcol = slice(dp * 128, (dp + 1) * 128)
                                for lh, rh in ((wqh_sb, xlth_sb), (wqh_sb, xltl_sb),
                                               (wql_sb, xlth_sb)):
                                    nc.tensor.matmul(pq[:], lh[:, es, dcol], rh[:, es, :],
                                                     start=(n_mm == 0), stop=(n_mm == 23))
                                    n_mm += 1
                            _split(nc, pq[:], qth[:, dp, :], qtl[:, dp, :])

                    with tc.tile_pool(name="w2", bufs=1) as w2:
                        wkth_sb = w2.tile([128, ET, D], F16, tag="wkth", name="wkth")
                        wktl_sb = w2.tile([128, ET, D], F16, tag="wktl", name="wktl")
                        for ds in range(ET):
                            r = slice(ds * 128, (ds + 1) * 128)
                            nc.sync.dma_start(wkth_sb[:, ds, :], wkth_d[r, :])
                            nc.sync.dma_start(wktl_sb[:, ds, :], wktl_d[r, :])

                        for ep in range(ET if _PHASES >= 2 else 0):
                            pc = psA.tile([128, NL], F32, tag="acc", name="acc")
                            n_mm = 0
                            for ds in range(ET):
                                ecol = slice(ep * 128, (ep + 1) * 128)
                                for lh, rh in ((wkth_sb, qth), (wkth_sb, qtl),
                                               (wktl_sb, qth)):
                                    nc.tensor.matmul(pc[:], lh[:, ds, ecol], rh[:, ds, :],
                                                     start=(n_mm == 0), stop=(n_mm == 23))
                                    n_mm += 1
                            _split(nc, pc[:], cth[:, ep, :], ctl[:, ep, :])

                # ---------------- P3: S chunks, chunk-max, fp16 store ----------------
                with tc.tile_pool(name="stream", bufs=2) as stream:
                    for jc in range(JC if _PHASES >= 3 else 0):
                        cols = slice(jc * 512, (jc + 1) * 512)
                        xch = stream.tile([128, ET, 512], F16, tag="xch", name="xch")
                        xcl = stream.tile([128, ET, 512], F16, tag="xcl", name="xcl")
                        nc.sync.dma_start(
                            xch[:], xth_d[:, cols].rearrange("(es p) j -> p es j", p=128))
                        nc.sync.dma_start(
                            xcl[:], xtl_d[:, cols].rearrange("(es p) j -> p es j", p=128))
                        # V rows for this chunk (used in P5), overlapped here
                        nc.sync.dma_start(
                            xh_sb[:, jc * 4:(jc + 1) * 4, :],
                            xh_d[jc * 512:(jc + 1) * 512, :]
                            .rearrange("(jt p) d -> p jt d", p=128))

                        for m in range(MT):
                            ps = psA.tile([128, 512], F32, tag="acc", name="acc")
                            mcol = slice(m * 128, (m + 1) * 128)
                            n_mm = 0
                            for es in range(ET):
                                for lh, rh in ((cth, xch), (cth, xcl), (ctl, xch)):
                                    nc.tensor.matmul(ps[:], lh[:, es, mcol], rh[:, es, :],
                                                     start=(n_mm == 0), stop=(n_mm == 23))
                                    n_mm += 1
                            nc.vector.reduce_max(pmax[m][:, jc:jc + 1], ps[:], axis=AX)
                            nc.vector.tensor_scalar_mul(
                                bsub[m][:, jc:jc + 1], pmax[m][:, jc:jc + 1], -SCALE)
                            nc.scalar.activation(s16[m][:, jc, :], ps[:], IDENT,
                                                 bias=bsub[m][:, jc:jc + 1], scale=SCALE)

            # ---------------- P4: global row max -> exp biases ----------------
            for m in range(MT if _PHASES >= 4 else 0):
                rowmax = stats.tile([128, 1], F32, tag=f"rmax{m}", name=f"rmax{m}")
                nc.vector.reduce_max(rowmax[:], pmax[m][:], axis=AX)
                # bexp = (pmax - rowmax) * SCALE
                nc.vector.tensor_scalar(bexp[m][:], pmax[m][:], rowmax[:], SCALE,
                                        op0=SUB, op1=MUL)

            # ---------------- P5: exp, P^T, P@V, scale ----------------
            with (
                tc.tile_pool(name="pvps", bufs=2, space=bass.MemorySpace.PSUM) as pvps,
                tc.tile_pool(name="ptps", bufs=3, space=bass.MemorySpace.PSUM) as ptps,
                tc.tile_pool(name="p5s", bufs=3) as p5s,
            ):
                for m in range(MT if _PHASES >= 5 else 0):
                    pv = pvps.tile([128, 2, 512], F32, tag="pv", name="pv")
                    for jc in range(JC):
                        pch = p5s.tile([128, 512], F16, tag="pch", name="pch")
                        nc.scalar.activation(pch[:], s16[m][:, jc, :], EXP,
                                             bias=bexp[m][:, jc:jc + 1], scale=1.0,
                                             accum_out=esum[m][:, jc:jc + 1])
                        ptt = ptps.tile([128, 4, 128], F16, tag="ptt", name="ptt")
                        for js in range(4):
                            nc.tensor.transpose(ptt[:, js, :],
                                                pch[:, js * 128:(js + 1) * 128], ident16[:])
                        pts = p5s.tile([128, 4, 128], F16, tag="pts", name="pts")
                        nc.vector.tensor_copy(pts[:], ptt[:])
                        for js in range(4):
                            jt = jc * 4 + js
                            for n in range(2):
                                nc.tensor.matmul(
                                    pv[:, n, :], pts[:, js, :],
                                    xh_sb[:, jt, n * 512:(n + 1) * 512],
                                    start=(jt == 0), stop=(jt == NT // 128 - 1))
                    rs = stats.tile([128, 1], F32, tag=f"rs{m}", name=f"rs{m}")
                    nc.vector.reduce_sum(rs[:], esum[m][:], axis=AX)
                    nc.vector.reciprocal(recip[:, m:m + 1], rs[:])
                    for n in range(2):
                        osb = p5s.tile([128, 512], F32, tag="osb", name="osb")
                        nc.vector.tensor_scalar_mul(osb[:], pv[:, n, :], recip[:, m:m + 1])
                        nc.sync.dma_start(
                            out_d[m * 128:(m + 1) * 128, n * 512:(n + 1) * 512], osb[:])

    nc.compile()
    return nc


_NC_CACHE = None


def _split16(a):
    h = a.astype(np.float16)
    l = (a - h.astype(np.float32)).astype(np.float16)
    return h, l


def kernel(inputs, rotation_params, entangle_params):
    global _NC_CACHE
    if _NC_CACHE is None:
        _NC_CACHE = build_nc()
    nc = _NC_CACHE

    x = np.ascontiguousarray(np.asarray(inputs, np.float32))
    wq = np.ascontiguousarray(np.asarray(rotation_params, np.float32))
    wkt = np.ascontiguousarray(np.asarray(entangle_params, np.float32).T)
    xt = np.ascontiguousarray(x.T)

    xh, _ = _split16(x)
    xth, xtl = _split16(xt)
    wqh, wql = _split16(wq)
    wkth, wktl = _split16(wkt)

    in_maps = []
    for c in range(NC):
        cols = slice(c * NL, (c + 1) * NL)
        in_maps.append({
            "xh": xh, "xth": xth, "xtl": xtl,
            "xlth": np.ascontiguousarray(xth[:, cols]),
            "xltl": np.ascontiguousarray(xtl[:, cols]),
            "wqh": wqh, "wql": wql, "wkth": wkth, "wktl": wktl,
        })
    r = run_bass_kernel_spmd(nc, in_maps, list(range(NC)))
    return np.concatenate([r.results[c]["out_local"] for c in range(NC)], axis=0)
